# revision 1
# baseline (speedup 1.0000x reference)
"""Trainium2 Bass kernel for nn_Block_21406117003497 (dense transformer block).

B=4, T=2048, C=1024, H=16 heads, HS=64, DFF=4096.
8 cores: core c -> batch c//2, token-half c%2 (causally balanced row split).
v2: fully transposed-domain dataflow. Host passes x^T; layernorm stats are
computed with ones-vector matmuls over the partition (channel) axis, so no
on-chip transposes are needed anywhere. Attention runs query-half (b) outer
so downstream proj/LN2 of half 0 overlaps the exp-bound attention of half 1.
Only Exp/Log activation functions are used (one table set, no reloads).
Matmuls in bf16, accumulation f32; residual path stays f32.
"""

import functools
from contextlib import ExitStack

import numpy as np
import ml_dtypes

import concourse.bass as bass
import concourse.mybir as mybir
import concourse.tile as tile
from concourse import bacc
from concourse.bass_utils import run_bass_kernel_spmd

F32 = mybir.dt.float32
BF16 = mybir.dt.bfloat16
AF = mybir.ActivationFunctionType
ALU = mybir.AluOpType
AX = mybir.AxisListType

B, T, C, H, HS = 4, 2048, 1024, 16, 64
DFF = 4 * C
R = 1024            # own rows per core
EPS = 1e-5
SCALE = float(C) ** -0.5
BF = ml_dtypes.bfloat16


def own_ranges(sub):
    """local row block -> absolute row ranges per sub (causally balanced)."""
    if sub == 0:
        return (0, 512), (1536, 2048)
    return (512, 1024), (1024, 1536)


def build_program(apply_ln_affine: bool, add_bproj: bool, add_b2: bool, repeat: int = 1,
                  loop_n: int = 0, variant: str = ""):
    nc = bacc.Bacc(None, target_bir_lowering=False, debug=False)

    env = {}
    env["variant"] = set(variant.split("+")) if variant else set()
    env["apply_ln_affine"] = apply_ln_affine
    env["add_bproj"] = add_bproj
    env["add_b2"] = add_b2
    env["xT_d"] = nc.dram_tensor("xT", [C, T], F32, kind="ExternalInput")
    env["xTq_d"] = nc.dram_tensor("xTq", [C, R], F32, kind="ExternalInput")
    env["maskp_d"] = nc.dram_tensor("maskp", [128, 16 * 512], BF16, kind="ExternalInput")
    env["wq_d"] = nc.dram_tensor("wq", [128, 8 * C], BF16, kind="ExternalInput")
    env["wk_d"] = nc.dram_tensor("wk", [128, 8 * C], BF16, kind="ExternalInput")
    env["wv_d"] = nc.dram_tensor("wv", [128, 8 * C], BF16, kind="ExternalInput")
    env["wp_d"] = nc.dram_tensor("wp", [128, 8 * C], BF16, kind="ExternalInput")
    env["w1_d"] = nc.dram_tensor("w1", [128, 8 * DFF], BF16, kind="ExternalInput")
    env["w2_d"] = nc.dram_tensor("w2", [128, 32 * C], BF16, kind="ExternalInput")
    env["b1r_d"] = nc.dram_tensor("b1r", [128, DFF // 128], F32, kind="ExternalInput")
    env["lnT_d"] = nc.dram_tensor("lnT", [128, 4 * 8], F32, kind="ExternalInput")
    env["bpT_d"] = nc.dram_tensor("bpT", [128, 2 * 8], F32, kind="ExternalInput")
    env["out_d"] = nc.dram_tensor("out", [C, R], F32, kind="ExternalOutput")

    with tile.TileContext(nc) as tc:
        with tc.tile_pool(name="consts", bufs=1, side="left") as consts:
            env["ones_t"] = consts.tile([128, 1], BF16, name="ones_t")
            nc.vector.memset(env["ones_t"][:], 1.0)
            env["eps_t"] = consts.tile([128, 1], F32, name="eps_t")
            nc.vector.memset(env["eps_t"][:], EPS)
            env["zeros_t"] = consts.tile([128, 512], F32, name="zeros_t")
            nc.vector.memset(env["zeros_t"][:], 0.0)
            env["b1r_t"] = consts.tile([128, DFF // 128], F32, name="b1r_t")
            nc.sync.dma_start(out=env["b1r_t"][:], in_=env["b1r_d"][:, :])
            if apply_ln_affine:
                lnT = consts.tile([128, 32], F32, name="lnT_t")
                nc.sync.dma_start(out=lnT[:], in_=env["lnT_d"][:, :])
                env["lnT_t"] = lnT
            if add_bproj or add_b2:
                bpT = consts.tile([128, 16], F32, name="bpT_t")
                nc.sync.dma_start(out=bpT[:], in_=env["bpT_d"][:, :])
                env["bpT_t"] = bpT

            if loop_n:
                with tc.For_i(0, loop_n, 1):
                    emit_block(nc, tc, env)
            else:
                for _rep in range(repeat):
                    emit_block(nc, tc, env)
    nc.compile()
    return nc


def emit_block(nc, tc, env):
    V = env["variant"]
    apply_ln_affine = env["apply_ln_affine"]
    xT_d, xTq_d, maskp_d = env["xT_d"], env["xTq_d"], env["maskp_d"]
    wq_d, wk_d, wv_d, wp_d = env["wq_d"], env["wk_d"], env["wv_d"], env["wp_d"]
    w1_d, w2_d = env["w1_d"], env["w2_d"]
    out_d = env["out_d"]
    ones_t, zeros_t, b1r_t = env["ones_t"], env["zeros_t"], env["b1r_t"]

    def ln_rows(ps_s, ps_q, row_p, arow_b, brow_b, nb):
        """stats psum [1,512] pair -> bf16 rstd / -mu*rstd rows at [0:1, nb-slice]."""
        sl = slice(nb * 512, nb * 512 + 512)
        mu = row_p.tile([1, 512], F32, name="mu", tag="rowtmp")
        nc.vector.tensor_scalar(out=mu[:], in0=ps_s[:], scalar1=1.0 / C,
                                scalar2=None, op0=ALU.mult)
        nmu2 = row_p.tile([1, 512], F32, name="nmu2", tag="rowtmp")
        nc.vector.scalar_tensor_tensor(nmu2[:], mu[:], -1.0, mu[:],
                                       op0=ALU.mult, op1=ALU.mult)
        var = row_p.tile([1, 512], F32, name="var", tag="rowtmp")
        nc.vector.scalar_tensor_tensor(var[:], ps_q[:], 1.0 / C, nmu2[:],
                                       op0=ALU.mult, op1=ALU.add)
        std = row_p.tile([1, 512], F32, name="std", tag="rowtmp")
        nc.scalar.activation(std[:], var[:], AF.Sqrt, bias=env["eps_t"][0:1, 0:1])
        rstd = row_p.tile([1, 512], F32, name="rstd", tag="rowtmp")
        nc.vector.reciprocal(rstd[:], std[:])
        nc.vector.tensor_copy(arow_b[0:1, sl], rstd[:])
        nc.vector.scalar_tensor_tensor(brow_b[0:1, sl], mu[:], -1.0, rstd[:],
                                       op0=ALU.mult, op1=ALU.mult)

    def ln_stats(src_loader, W, ps_pool, pools, sfx, ps_tag=None):
        """Pass 1: load+cast 8 chunks, packed colsum/colsumsq stats matmuls.

        Returns (xb chunk list, packed stats psum tile [2*nbs, 512]).
        """
        nbs = W // 512
        p_xb, p_sq = pools
        tg = ps_tag or f"st{sfx}"
        st_s = [ps_pool.tile([1, 512], F32, name=f"st{sfx}_s{nb}", tag=tg)
                for nb in range(nbs)]
        st_q = [ps_pool.tile([1, 512], F32, name=f"st{sfx}_q{nb}", tag=tg)
                for nb in range(nbs)]
        xb = []
        for c in range(8):
            xt = src_loader(c)
            xbc = p_xb.tile([128, W], BF16, name=f"xb{sfx}{c}", tag=f"xb{sfx}")
            nc.vector.tensor_copy(xbc[:], xt[:])
            sq = p_sq.tile([128, W], BF16, name=f"sq{sfx}", tag=f"sq{sfx}")
            nc.scalar.activation(sq[:], xbc[:], AF.Square)
            for nb in range(nbs):
                sl = slice(nb * 512, nb * 512 + 512)
                nc.tensor.matmul(st_s[nb][:], ones_t[:], xbc[:, sl],
                                 start=(c == 0), stop=(c == 7))
                nc.tensor.matmul(st_q[nb][:], ones_t[:], sq[:, sl],
                                 start=(c == 0), stop=(c == 7))
            xb.append(xbc)
        return xb, (st_s, st_q)

    def ln_normalize(xb, st, W, dst_writer, pools, gcol, bcol, sfx):
        """Pass 2: rows from packed stats, broadcast, normalize into dst."""
        nbs = W // 512
        p_sq, p_row, p_ab, p_bc = pools
        st_s, st_q = st
        arow_b = p_ab.tile([1, W], BF16, name=f"arow_b{sfx}", tag=f"ab_{sfx}")
        brow_b = p_ab.tile([1, W], BF16, name=f"brow_b{sfx}", tag=f"ab_{sfx}")
        for nb in range(nbs):
            ln_rows(st_s[nb], st_q[nb], p_row, arow_b, brow_b, nb)
        ab = p_bc.tile([128, W], BF16, name=f"ab{sfx}", tag=f"bc_{sfx}")
        bb = p_bc.tile([128, W], BF16, name=f"bb{sfx}", tag=f"bc_{sfx}")
        nc.gpsimd.partition_broadcast(ab[:], arow_b[0:1, :])
        nc.gpsimd.partition_broadcast(bb[:], brow_b[0:1, :])
        for c in range(8):
            tmp = p_sq.tile([128, W], BF16, name=f"ntmp{sfx}", tag=f"sq{sfx}")
            # alternate the mul between POOL and DVE to balance engine load
            eng = nc.gpsimd if c % 2 == 0 else nc.vector
            eng.tensor_mul(tmp[:], xb[c][:], ab[:])
            dst = dst_writer(c)
            nc.vector.tensor_add(dst, tmp[:], bb[:])
            if apply_ln_affine:
                nc.vector.tensor_scalar(out=dst, in0=dst,
                                        scalar1=gcol(c), scalar2=bcol(c),
                                        op0=ALU.mult, op1=ALU.add)

    def ln_cols(kind, c):
        lnT = env["lnT_t"]
        return lnT[:, kind * 8 + c: kind * 8 + c + 1]

    # ================= Phase A: LN1 -> hT (full T) and hTq (own rows) ========
    es_h = ExitStack()
    p_hT = es_h.enter_context(tc.tile_pool(name="p_hT", bufs=8, side="left"))
    p_hTq = es_h.enter_context(tc.tile_pool(name="p_hTq", bufs=8, side="left"))
    hT = [p_hT.tile([128, T], BF16, name=f"hT{c}", tag="hT") for c in range(8)]
    hTq = [p_hTq.tile([128, R], BF16, name=f"hTq{c}", tag="hTq") for c in range(8)]

    if "noln" in V:
        for t_ in hT + hTq:
            nc.vector.memset(t_[:, :], 0.5)
    else:
        with tc.tile_pool(name="ps_st", bufs=8, space="PSUM") as ps_st, \
             tc.tile_pool(name="p_xin", bufs=3, side="right") as p_xin, \
             tc.tile_pool(name="p_lnxb", bufs=8, side="right") as p_xb, \
             tc.tile_pool(name="p_lnsq", bufs=3, side="right") as p_sq, \
             tc.tile_pool(name="p_lnrow", bufs=3, side="right") as p_row, \
             tc.tile_pool(name="p_lnab", bufs=2, side="right") as p_ab, \
             tc.tile_pool(name="p_lnbc", bufs=2, side="right") as p_bc, \
             tc.tile_pool(name="p_lnxbq", bufs=8, side="left") as p_xbq, \
             tc.tile_pool(name="p_lnsqq", bufs=3, side="left") as p_sqq, \
             tc.tile_pool(name="p_lnbcq", bufs=2, side="left") as p_bcq:
            def load_kv(c):
                xt = p_xin.tile([128, T], F32, name="xkv", tag="xin")
                nc.sync.dma_start(out=xt[:], in_=xT_d[c * 128:(c + 1) * 128, :])
                return xt

            def load_q(c):
                xt = p_xin.tile([128, R], F32, name="xq", tag="xinq")
                nc.sync.dma_start(out=xt[:], in_=xTq_d[c * 128:(c + 1) * 128, :])
                return xt

            g1 = lambda c: ln_cols(0, c)
            b1 = lambda c: ln_cols(1, c)
            xb1, st1 = ln_stats(load_kv, T, ps_st, (p_xb, p_sq), "")
            xbq, stq = ln_stats(load_q, R, ps_st, (p_xbq, p_sqq), "q", ps_tag="st")
            ln_normalize(xb1, st1, T, lambda c: hT[c][:],
                         (p_sq, p_row, p_ab, p_bc), g1, b1, "")
            ln_normalize(xbq, stq, R, lambda c: hTq[c][:],
                         (p_sqq, p_row, p_ab, p_bcq), g1, b1, "q")

    # ================= Phase B: QKV projections =============================
    es_qkv = ExitStack()
    p_QT = es_qkv.enter_context(tc.tile_pool(name="p_QT", bufs=16, side="right"))
    p_KT = es_qkv.enter_context(tc.tile_pool(name="p_KT", bufs=8, side="right"))
    p_V = es_qkv.enter_context(tc.tile_pool(name="p_V", bufs=16, side="right"))
    QTb = [[p_QT.tile([128, 512], BF16, name=f"QT{b}_{m}", tag="QT")
            for m in range(8)] for b in range(2)]
    KT = [p_KT.tile([128, T], BF16, name=f"KT{m}", tag="KT") for m in range(8)]
    Vg = [p_V.tile([128, 16 * 65], BF16, name=f"Vg{i}", tag="Vg") for i in range(16)]

    with tc.tile_pool(name="p_w", bufs=2, side="right") as p_w, \
         tc.tile_pool(name="ps_qkv", bufs=4, space="PSUM") as ps_a:
        if "noqkv" in V:
            for t_ in QTb[0] + QTb[1] + KT + Vg:
                nc.vector.memset(t_[:, :], 0.01)
        else:
            # K projection (full T)
            wt = p_w.tile([128, 8 * C], BF16, name="w_k", tag="wsb")
            nc.sync.dma_start(out=wt[:], in_=wk_d[:, :])
            for m in range(8):
                for nbp in range(2):
                    psA = ps_a.tile([128, 512], F32, name="qk_psA", tag="ps_a")
                    psB = ps_a.tile([128, 512], F32, name="qk_psB", tag="ps_a")
                    for k in range(8):
                        lhs = wt[:, k * C + m * 128:k * C + (m + 1) * 128]
                        nc.tensor.matmul(psA[:], lhs,
                                         hT[k][:, (2 * nbp) * 512:(2 * nbp + 1) * 512],
                                         start=(k == 0), stop=(k == 7))
                        nc.tensor.matmul(psB[:], lhs,
                                         hT[k][:, (2 * nbp + 1) * 512:(2 * nbp + 2) * 512],
                                         start=(k == 0), stop=(k == 7))
                    nc.any.tensor_copy(KT[m][:, (2 * nbp) * 512:(2 * nbp + 1) * 512], psA[:])
                    nc.any.tensor_copy(KT[m][:, (2 * nbp + 1) * 512:(2 * nbp + 2) * 512], psB[:])
            # Q projection (own rows, split by query-half b)
            wt = p_w.tile([128, 8 * C], BF16, name="w_q", tag="wsb")
            nc.sync.dma_start(out=wt[:], in_=wq_d[:, :])
            for m in range(8):
                psA = ps_a.tile([128, 512], F32, name="qk_psA", tag="ps_a")
                psB = ps_a.tile([128, 512], F32, name="qk_psB", tag="ps_a")
                for k in range(8):
                    lhs = wt[:, k * C + m * 128:k * C + (m + 1) * 128]
                    nc.tensor.matmul(psA[:], lhs, hTq[k][:, 0:512],
                                     start=(k == 0), stop=(k == 7))
                    nc.tensor.matmul(psB[:], lhs, hTq[k][:, 512:1024],
                                     start=(k == 0), stop=(k == 7))
                nc.any.tensor_copy(QTb[0][m][:], psA[:])
                nc.any.tensor_copy(QTb[1][m][:], psB[:])

            wt = p_w.tile([128, 8 * C], BF16, name="w_v", tag="wsb")
            nc.sync.dma_start(out=wt[:], in_=wv_d[:, :])
            for tch in range(16):
                nc.gpsimd.memset(
                    Vg[tch][:, 0:16 * 65].rearrange("p (h d) -> p h d", d=65)[:, :, 64:65], 1.0)
                psA = ps_a.tile([128, 512], F32, name="v_psA", tag="ps_a")
                psB = ps_a.tile([128, 512], F32, name="v_psB", tag="ps_a")
                for k in range(8):
                    lhs = hT[k][:, tch * 128:(tch + 1) * 128]
                    nc.tensor.matmul(psA[:], lhs, wt[:, k * C:k * C + 512],
                                     start=(k == 0), stop=(k == 7))
                    nc.tensor.matmul(psB[:], lhs, wt[:, k * C + 512:k * C + 1024],
                                     start=(k == 0), stop=(k == 7))
                for j, ps in ((0, psA), (1, psB)):
                    dst = Vg[tch][:, j * 8 * 65:(j + 1) * 8 * 65].rearrange(
                        "p (h d) -> p h d", d=65)[:, :, 0:64]
                    nc.any.tensor_copy(dst, ps[:].rearrange("p (h d) -> p h d", d=64))
    es_h.close()  # hT/hTq freed (left side)

    # ================= Phase C: attention (b outer) + proj + LN2 ============
    es_x2 = ExitStack()
    p_x2 = es_x2.enter_context(tc.tile_pool(name="p_x2", bufs=8, side="left"))
    p_h2T = es_x2.enter_context(tc.tile_pool(name="p_h2T", bufs=8, side="left"))
    x2T = [p_x2.tile([128, R], F32, name=f"x2_{c}", tag="x2") for c in range(8)]
    h2T = [p_h2T.tile([128, R], BF16, name=f"h2T{c}", tag="h2T") for c in range(8)]

    LAG = 4
    es_att = ExitStack()
    p_oT = es_att.enter_context(tc.tile_pool(name="p_oT", bufs=16, side="left"))
    p_mask = es_att.enter_context(tc.tile_pool(name="p_mask", bufs=8, side="right"))
    p_E = es_att.enter_context(tc.tile_pool(name="p_E", bufs=6, side="right"))
    p_inv = es_att.enter_context(tc.tile_pool(name="p_inv", bufs=2, side="right"))
    ps_s = es_att.enter_context(tc.tile_pool(name="ps_s", bufs=2, space="PSUM"))
    ps_av = es_att.enter_context(tc.tile_pool(name="ps_av", bufs=2, space="PSUM"))
    ps_pj = es_att.enter_context(tc.tile_pool(name="ps_pj", bufs=2, space="PSUM"))

    oTb = [[p_oT.tile([128, 512], BF16, name=f"oT{b}_{m}", tag="oT")
            for m in range(8)] for b in range(2)]

    def attn_b(b):
        n_kt = 8 if b == 0 else 16
        mk = []
        for mi in range(8):
            mt = p_mask.tile([128, 512], BF16, name=f"mk{mi}", tag="mk")
            nc.sync.dma_start(out=mt[:],
                              in_=maskp_d[:, (b * 8 + mi) * 512:(b * 8 + mi + 1) * 512])
            mk.append(mt)
        for hp in range(8):
            qslc = [QTb[b][hp][hh * 64:(hh + 1) * 64, :] for hh in (0, 1)]
            oa = ps_av.tile([128, 512], F32, name="av_psA", tag="ps_av")
            ob = ps_av.tile([128, 512], F32, name="av_psB", tag="ps_av")
            Es = []
            for kt in range(n_kt + LAG):
                if kt < n_kt:
                    masked = (b == 0) or (kt >= 8)
                    sps = ps_s.tile([128, 1024], F32, name="s_ps", tag="ps_s")
                    for hh in (0, 1):
                        kslc = KT[hp][hh * 64:(hh + 1) * 64, kt * 128:(kt + 1) * 128]
                        nc.tensor.matmul(sps[:, hh * 512:(hh + 1) * 512],
                                         kslc, qslc[hh], start=True, stop=True)
                    E = p_E.tile([128, 1024], BF16, name="E", tag="E")
                    nc.scalar.activation(E[:], sps[:], AF.Exp, scale=SCALE)
                    if masked and "nomask" not in V:
                        mi = kt if b == 0 else kt - 8
                        e3 = E[:, 0:1024].rearrange("p (h r) -> p h r", r=512)
                        m3 = mk[mi][:, None, :].broadcast_to([128, 2, 512])
                        nc.vector.tensor_mul(e3, e3, m3)
                    Es.append(E)
                kta = kt - LAG
                if 0 <= kta:
                    h0 = 2 * hp
                    nc.tensor.matmul(oa[0:65, :],
                                     Vg[kta][:, h0 * 65:h0 * 65 + 65],
                                     Es[kta][:, 0:512],
                                     start=(kta == 0), stop=(kta == n_kt - 1))
                    nc.tensor.matmul(ob[0:65, :],
                                     Vg[kta][:, (h0 + 1) * 65:(h0 + 1) * 65 + 65],
                                     Es[kta][:, 512:1024],
                                     start=(kta == 0), stop=(kta == n_kt - 1))
            for hh, ops in ((0, oa), (1, ob)):
                invd = p_inv.tile([1, 512], F32, name="invd", tag="invd")
                nc.vector.reciprocal(invd[:], ops[64:65, :])
                invb = p_inv.tile([64, 512], F32, name="invb", tag="invb")
                nc.gpsimd.partition_broadcast(invb[:], invd[0:1, :])
                nc.vector.tensor_mul(oTb[b][hp][hh * 64:(hh + 1) * 64, :],
                                     ops[0:64, :], invb[:])

    def proj_ln2_b(b):
        bsl = slice(b * 512, (b + 1) * 512)
        # proj^T + residual -> x2T[:, b-half]
        with tc.tile_pool(name="p_xo", bufs=3, side="right") as p_xo, \
             tc.tile_pool(name="p_wp", bufs=2, side="right") as p_wp:
            for m in range(8):
                wpt = p_wp.tile([128, C], BF16, name="wp_t", tag="wp")
                nc.sync.dma_start(out=wpt[:], in_=wp_d[:, m * C:(m + 1) * C])
                xo = p_xo.tile([128, 512], F32, name="xo", tag="xo")
                nc.sync.dma_start(out=xo[:], in_=xTq_d[m * 128:(m + 1) * 128, bsl])
                ps = ps_pj.tile([128, 512], F32, name="pj_ps", tag="ps_pj")
                for k in range(8):
                    lhs = wpt[:, k * 128:(k + 1) * 128]
                    nc.tensor.matmul(ps[:], lhs, oTb[b][k][:, :],
                                     start=(k == 0), stop=(k == 7))
                nc.vector.scalar_tensor_tensor(x2T[m][:, bsl], ps[:], 1.0, xo[:],
                                               op0=ALU.mult, op1=ALU.add)
                if env["add_bproj"]:
                    nc.vector.tensor_scalar(out=x2T[m][:, bsl], in0=x2T[m][:, bsl],
                                            scalar1=env["bpT_t"][:, m:m + 1],
                                            scalar2=None, op0=ALU.add)
        # LN2 on x2T[:, b-half] -> h2T[:, b-half]
        with tc.tile_pool(name="p_l2b", bufs=8, side="right") as p_xb2, \
             tc.tile_pool(name="p_l2sq", bufs=3, side="right") as p_sq2, \
             tc.tile_pool(name="p_l2row", bufs=3, side="right") as p_row2, \
             tc.tile_pool(name="p_l2ab", bufs=2, side="right") as p_ab2, \
             tc.tile_pool(name="p_l2bc", bufs=2, side="right") as p_bc2:
            def load_x2(c):
                return x2T[c][:, bsl]
            xb2, st2 = ln_stats(load_x2, 512, ps_pj, (p_xb2, p_sq2), "2",
                                ps_tag="ps_pj")
            ln_normalize(xb2, st2, 512, lambda c: h2T[c][:, bsl],
                         (p_sq2, p_row2, p_ab2, p_bc2),
                         lambda c: ln_cols(2, c), lambda c: ln_cols(3, c), "2")

    if "noattn" in V:
        for bl in oTb:
            for t_ in bl:
                nc.vector.memset(t_[:, :], 0.01)
    else:
        attn_b(0)
        attn_b(1)
    if "noproj" in V:
        for t_ in x2T:
            nc.vector.memset(t_[:, :], 0.01)
        for t_ in h2T:
            nc.vector.memset(t_[:, :], 0.5)
    else:
        proj_ln2_b(0)
        proj_ln2_b(1)
    es_att.close()
    es_qkv.close()  # QT/KT/Vg freed before MLP needs the space

    # ================= Phase D: MLP =========================================
    es_r1 = ExitStack()
    p_r1 = es_r1.enter_context(tc.tile_pool(name="p_r1", bufs=32, side="right"))
    r1T = [p_r1.tile([128, R], BF16, name=f"r1T{g}", tag="r1T") for g in range(32)]
    ps_m = es_r1.enter_context(tc.tile_pool(name="ps_m", bufs=4, space="PSUM"))

    with tc.tile_pool(name="p_w1", bufs=3, side="left") as p_w1, \
         tc.tile_pool(name="p_w2", bufs=2, side="left") as p_w2:
        for db in (() if "nomlp" in V else range(8)):
            w1t = p_w1.tile([128, DFF], BF16, name=f"w1t{db}", tag="w1t")
            nc.sync.dma_start(out=w1t[:], in_=w1_d[:, db * DFF:(db + 1) * DFF])
            for dc in range(4):
                g = db * 4 + dc
                psA = ps_m.tile([128, 512], F32, name="m1_psA", tag="ps_m")
                psB = ps_m.tile([128, 512], F32, name="m1_psB", tag="ps_m")
                for k in range(8):
                    lhs = w1t[:, k * 512 + dc * 128:k * 512 + (dc + 1) * 128]
                    nc.tensor.matmul(psA[:], lhs, h2T[k][:, 0:512],
                                     start=(k == 0), stop=(k == 7))
                    nc.tensor.matmul(psB[:], lhs, h2T[k][:, 512:1024],
                                     start=(k == 0), stop=(k == 7))
                for j, ps in ((0, psA), (1, psB)):
                    nc.vector.scalar_tensor_tensor(
                        r1T[g][:, j * 512:(j + 1) * 512], ps[:], b1r_t[:, g:g + 1],
                        zeros_t[:], op0=ALU.add, op1=ALU.max)

        for q in range(4) if "nomlp" not in V else ():
            w2t = p_w2.tile([128, 8 * C], BF16, name=f"w2t{q}", tag="w2t")
            nc.sync.dma_start(out=w2t[:], in_=w2_d[:, q * 8 * C:(q + 1) * 8 * C])
            for m in range(8):
                for rb in range(2):
                    sl = slice(rb * 512, (rb + 1) * 512)
                    ps = ps_m.tile([128, 512], F32, name="m2_ps", tag="ps_m")
                    for kl in range(8):
                        lhs = w2t[:, kl * C + m * 128:kl * C + (m + 1) * 128]
                        nc.tensor.matmul(ps[:], lhs, r1T[q * 8 + kl][:, sl],
                                         start=(kl == 0), stop=(kl == 7))
                    nc.vector.scalar_tensor_tensor(x2T[m][:, sl], ps[:], 1.0,
                                                   x2T[m][:, sl], op0=ALU.mult, op1=ALU.add)
                if q == 3:
                    if env["add_b2"]:
                        nc.vector.tensor_scalar(out=x2T[m][:, :], in0=x2T[m][:, :],
                                                scalar1=env["bpT_t"][:, 8 + m:8 + m + 1],
                                                scalar2=None, op0=ALU.add)
                    nc.sync.dma_start(out=out_d[m * 128:(m + 1) * 128, :], in_=x2T[m][:])

    es_r1.close()
    es_x2.close()


@functools.lru_cache(maxsize=16)
def _cached_program(apply_ln_affine, add_bproj, add_b2, repeat, loop_n=0, variant=""):
    return build_program(apply_ln_affine, add_bproj, add_b2, repeat, loop_n, variant)


def _pack_rows(w):
    """[8k*128, N] -> [128, 8k*N] with col = k*N + j."""
    kchunks = w.shape[0] // 128
    return np.ascontiguousarray(
        w.reshape(kchunks, 128, w.shape[1]).transpose(1, 0, 2).reshape(128, -1)
    ).astype(BF)


def _prep_shards(x, Wq, Wk, Wv, Wproj, bproj, ln1_g, ln1_b, ln2_g, ln2_b, W1, b1, W2, b2):
    wq = _pack_rows(Wq.transpose(1, 0, 2).reshape(C, C))
    wk = _pack_rows(Wk.transpose(1, 0, 2).reshape(C, C))
    wv = _pack_rows(Wv.transpose(1, 0, 2).reshape(C, C))
    # wp: m-major pack: [128, m*1024 + k*128 + j] = Wproj[k*128+p, m*128+j]
    wp = np.ascontiguousarray(
        np.asarray(Wproj).reshape(8, 128, 8, 128).transpose(1, 2, 0, 3).reshape(128, C * 8)
    ).astype(BF)
    # w1: [128, db*4096 + k*512 + jj] = W1[k*128+p, db*512+jj]
    w1 = np.ascontiguousarray(
        np.asarray(W1).reshape(8, 128, 8, 512).transpose(1, 2, 0, 3).reshape(128, 8 * DFF)
    ).astype(BF)
    w2 = _pack_rows(np.asarray(W2))
    b1r = np.ascontiguousarray(np.asarray(b1).reshape(DFF // 128, 128).T).astype(np.float32)
    lnT = np.ascontiguousarray(
        np.stack([ln1_g, ln1_b, ln2_g, ln2_b]).reshape(4, 8, 128).transpose(2, 0, 1)
        .reshape(128, 32)).astype(np.float32)
    bpT = np.ascontiguousarray(
        np.stack([bproj, b2]).reshape(2, 8, 128).transpose(2, 0, 1).reshape(128, 16)
    ).astype(np.float32)

    in_maps = []
    for c in range(8):
        bidx, sub = c // 2, c % 2
        (lo0, lo1), (hi0, hi1) = own_ranges(sub)
        xb = np.asarray(x[bidx])
        x_own = np.concatenate([xb[lo0:lo1], xb[hi0:hi1]], axis=0)
        keys = np.arange(T)
        rows_b0 = np.arange(lo0, lo1)
        rows_b1 = np.arange(hi0, hi1)
        m = np.zeros((T, 512), np.float32)
        m[0:1024] = (keys[0:1024, None] <= rows_b0[None, :])
        m[1024:2048] = (keys[1024:2048, None] <= rows_b1[None, :])
        maskp = np.ascontiguousarray(
            m.reshape(16, 128, 512).transpose(1, 0, 2).reshape(128, 16 * 512)).astype(BF)
        in_maps.append({
            "xT": np.ascontiguousarray(xb.T).astype(np.float32),
            "xTq": np.ascontiguousarray(x_own.T).astype(np.float32),
            "maskp": maskp,
            "wq": wq, "wk": wk, "wv": wv, "wp": wp,
            "w1": w1, "w2": w2, "b1r": b1r, "lnT": lnT, "bpT": bpT,
        })
    return in_maps


def kernel(repeat: int = 1, loop_n: int = 0, variant: str = "", **inputs) -> np.ndarray:
    inputs = {k: np.asarray(v) for k, v in inputs.items()}
    apply_ln_affine = not (
        np.all(inputs["ln1_g"] == 1) and np.all(inputs["ln1_b"] == 0)
        and np.all(inputs["ln2_g"] == 1) and np.all(inputs["ln2_b"] == 0))
    add_bproj = bool(np.any(inputs["bproj"] != 0))
    add_b2 = bool(np.any(inputs["b2"] != 0))
    nc = _cached_program(apply_ln_affine, add_bproj, add_b2, repeat, loop_n, variant)
    in_maps = _prep_shards(**inputs)
    res = run_bass_kernel_spmd(nc, in_maps, list(range(8)))
    out = np.empty((B, T, C), np.float32)
    for c in range(8):
        bidx, sub = c // 2, c % 2
        (lo0, lo1), (hi0, hi1) = own_ranges(sub)
        oc = np.asarray(res.results[c]["out"]).T  # [R, C]
        out[bidx, lo0:lo1] = oc[0:512]
        out[bidx, hi0:hi1] = oc[512:1024]
    return out

